# revision 24
# baseline (speedup 1.0000x reference)
"""Causal multi-head attention on 8 Trainium2 NeuronCores.

Problem: B=2, S=2048, E=1024, H=16 heads, D=64.
Sharding: core c handles batch b = c // 4 and heads [4*(c%4) .. 4*(c%4)+3]
(data parallel on B x tensor parallel on heads). Each core computes its
partial output projection; the host sums the 4 partials per batch and adds
b_proj (the standard row-parallel TP reduction, done on host).

Per-core kernel (all matmuls bf16 inputs, fp32 PSUM accumulation), computed
in "transposed" space to avoid transposing softmax probabilities:
  xT = X[b]^T in SBUF.
  Q^T/K^T [D, S] per head via col-packed matmuls (Q in psum rows 0:64, K in
  rows 64:128), evicted with ACT (bias + 1/8 scaling folded into Q).
  V [S, D] per head with a ones column appended -> AV matmul also
  accumulates softmax denominators.
  Scores S^T [k, q] = K^T.T @ Q^T, row-packed across head pairs.
  P^T = exp(S^T) on ACT; diagonal blocks masked with precomputed causal
  masks; strictly-upper blocks skipped.
  A^T [65, q] = V_ext.T @ P^T accumulated over k tiles (row 64 = denom).
  1/denom broadcast across partitions via a K=1 outer-product matmul,
  divide on DVE into head-pair-stacked A_scaled^T.
  partial[q, e] = A_scaled^T.T @ W_proj_rows accumulated over both pairs.
"""

import os
import sys
from contextlib import ExitStack

for _p in ("/opt/trn_rl_repo", "/root/.axon_site/_ro/trn_rl_repo"):
    if os.path.isdir(_p) and _p not in sys.path:
        sys.path.append(_p)

import numpy as np
import ml_dtypes

import concourse.bass as bass
import concourse.tile as tile
from concourse import bacc
from concourse import mybir
from concourse.masks import make_identity  # noqa: F401  (import check)

FP32 = mybir.dt.float32
BF16 = mybir.dt.bfloat16
AF = mybir.ActivationFunctionType

B, S, E, H = 2, 2048, 1024, 16
D = E // H          # 64
NCORES = 8
HPC = 4             # heads per core
NPAIR = 2           # head pairs per core
KT = E // 128       # 8 contraction tiles over E
ST = S // 128       # 16 tiles over S (k dimension)
QM = S // 512       # 4 q-macro tiles of 512
NQ = 512


def build_graph():
    nc = bacc.Bacc()

    xT = nc.declare_dram_parameter("xT", [E, S], BF16, isOutput=False)
    wq = nc.declare_dram_parameter("wq", [E, HPC * D], BF16, isOutput=False)
    wk = nc.declare_dram_parameter("wk", [E, HPC * D], BF16, isOutput=False)
    wv = nc.declare_dram_parameter("wv", [E, HPC * D], BF16, isOutput=False)
    qkbias = nc.declare_dram_parameter("qkbias", [128, HPC], FP32, isOutput=False)
    bv = nc.declare_dram_parameter("bv", [1, HPC * D], BF16, isOutput=False)
    wp = nc.declare_dram_parameter("wp", [HPC * D, E], BF16, isOutput=False)
    out = nc.declare_dram_parameter("out", [S, E], FP32, isOutput=True)

    with tile.TileContext(nc) as tc, ExitStack() as ctx:
        const = ctx.enter_context(tc.tile_pool(name="const", bufs=1))
        sb = ctx.enter_context(tc.tile_pool(name="sb", bufs=1))
        pexp_pool = ctx.enter_context(tc.tile_pool(name="pexp", bufs=8))
        stage = ctx.enter_context(tc.tile_pool(name="stage", bufs=3))
        rec_pool = ctx.enter_context(tc.tile_pool(name="rec", bufs=2))
        araw_pool = ctx.enter_context(tc.tile_pool(name="araw", bufs=2))

        # PSUM budget is 8 banks: scores 4 + qkv/proj/bcast 2 + psa0/psa1 2
        ps_s = ctx.enter_context(tc.tile_pool(name="ps_s", bufs=4, space="PSUM"))
        ps_qkv = ctx.enter_context(tc.tile_pool(name="ps_qkv", bufs=2, space="PSUM"))
        ps_a = ctx.enter_context(tc.tile_pool(name="ps_a", bufs=1, space="PSUM"))

        # ---- persistent SBUF tensors ----
        xt_sb = sb.tile([128, KT, S], BF16)          # X^T tiles, kt-major
        qt_sb = sb.tile([128, NPAIR, S], BF16)       # Q^T, pair-stacked
        kt_sb = sb.tile([128, NPAIR, S], BF16)       # K^T, pair-stacked
        v_sb = sb.tile([128, ST, HPC, D + 1], BF16)  # [V | ones] per ktile/head
        as_sb = sb.tile([128, NPAIR, S], BF16)       # A_scaled^T, pair-stacked
        wq_sb = sb.tile([128, KT, HPC * D], BF16)
        wk_sb = sb.tile([128, KT, HPC * D], BF16)
        wv_sb = sb.tile([128, KT, HPC * D], BF16)
        wp_sb = sb.tile([128, NPAIR, E], BF16)
        qkb_sb = const.tile([128, HPC], FP32)
        bv_sb = const.tile([1, HPC * D], BF16)
        ones_row = const.tile([1, 128], BF16)        # K=1 lhsT for V bias
        ones97 = const.tile([97, D], BF16)           # rows 0/32/64/96: K=1 lhsT
        tri = const.tile([128, 128], BF16)           # upper-tri (incl diag) strip mask

        # ---- constants ----
        nc.any.memset(ones_row[:], 1.0)
        for _h in range(4):
            nc.any.memset(ones97[32 * _h:32 * _h + 1, :], 1.0)
        nc.any.memset(v_sb[:, :, :, D:D + 1], 1.0)
        # tri[kk, qq] = 1 where kk <= qq else 0
        nc.any.memset(tri[:], 1.0)
        nc.gpsimd.affine_select(
            out=tri[:], in_=tri[:],
            compare_op=mybir.AluOpType.is_ge, fill=0.0,
            base=0, pattern=[[1, 128]], channel_multiplier=-1)

        # ---- input DMAs: critical-first, alternating the two HWDGE rings ----
        _dq = [nc.sync, nc.scalar]
        _di = [0]

        def dma_in(dst, src):
            _dq[_di[0] % 2].dma_start(dst, src)
            _di[0] += 1

        dma_in(qkb_sb[:], qkbias[:])
        dma_in(bv_sb[:], bv[:])
        for kt in range(KT):
            dma_in(wv_sb[:, kt, :], wv[kt * 128:(kt + 1) * 128, :])
            dma_in(xt_sb[:, kt, 0:NQ], xT[kt * 128:(kt + 1) * 128, 0:NQ])
        for kt in range(KT):
            dma_in(wq_sb[:, kt, :], wq[kt * 128:(kt + 1) * 128, :])
            dma_in(wk_sb[:, kt, :], wk[kt * 128:(kt + 1) * 128, :])
        for _c in range(1, QM):
            for kt in range(KT):
                dma_in(xt_sb[:, kt, _c * NQ:(_c + 1) * NQ],
                       xT[kt * 128:(kt + 1) * 128, _c * NQ:(_c + 1) * NQ])
            if _c == 1:
                dma_in(wp_sb[:, 0, :], wp[0:128, :])
                dma_in(wp_sb[:, 1, :], wp[128:256, :])

        def qkv_stage(m):
            msl = slice(m * NQ, (m + 1) * NQ)
            # V projection for this macro's 4 s-tiles
            for st in range(4 * m, 4 * m + 4):
                psv = ps_qkv.tile([128, NQ], FP32, name="psv", tag="qkv")
                ssl = slice(st * 128, (st + 1) * 128)
                for kt in range(KT):
                    nc.tensor.matmul(
                        psv[:, 0:HPC * D], lhsT=xt_sb[:, kt, ssl],
                        rhs=wv_sb[:, kt, :], start=(kt == 0), stop=False)
                nc.tensor.matmul(
                    psv[:, 0:HPC * D], lhsT=ones_row[:], rhs=bv_sb[:],
                    start=False, stop=True)
                nc.vector.tensor_copy(
                    v_sb[:, st, :, 0:D],
                    psv[:, 0:HPC * D].rearrange("p (h d) -> p h d", h=HPC))
            # Q^T / K^T for this macro, col-packed (Q rows 0:64, K rows 64:128)
            for p in range(NPAIR):
                for hh in range(2):
                    h = 2 * p + hh
                    lo, hi = hh * 64, hh * 64 + 64
                    psqk = ps_qkv.tile([128, NQ], FP32, name="psqk", tag="qkv")
                    for kt in range(KT):
                        nc.tensor.matmul(
                            psqk[0:64, :],
                            lhsT=wq_sb[:, kt, h * D:(h + 1) * D],
                            rhs=xt_sb[:, kt, msl],
                            start=(kt == 0), stop=(kt == KT - 1),
                            tile_position=(0, 0), skip_group_check=True)
                        nc.tensor.matmul(
                            psqk[64:128, :],
                            lhsT=wk_sb[:, kt, h * D:(h + 1) * D],
                            rhs=xt_sb[:, kt, msl],
                            start=(kt == 0), stop=(kt == KT - 1),
                            tile_position=(0, 64), skip_group_check=True)
                    nc.vector.tensor_scalar(
                        qt_sb[lo:hi, p, msl], psqk[0:64, :],
                        0.125, qkb_sb[0:64, h:h + 1],
                        op0=mybir.AluOpType.mult, op1=mybir.AluOpType.add)
                    nc.vector.tensor_scalar_add(
                        kt_sb[lo:hi, p, msl], psqk[64:128, :],
                        qkb_sb[64:128, h:h + 1])
        qkv_stage(0)
        for m in range(QM):
            msl = slice(m * NQ, (m + 1) * NQ)
            coll = rec_pool.tile([97, NQ], FP32, name="coll", tag="coll")
            nc.any.memset(coll[:], 1.0)
            recf = rec_pool.tile([97, NQ], FP32, name="recf", tag="recf")
            recb = rec_pool.tile([97, NQ], BF16, name="recb", tag="recb")
            araw = [araw_pool.tile([64, NQ], BF16, name=f"araw{_h}",
                                   tag=f"araw{_h}") for _h in range(HPC)]
            for p in range(NPAIR):
                psa = [ps_a.tile([65, NQ], FP32, name=f"psa{_hh}",
                                 tag=f"psa{_hh}")
                       for _hh in range(2)]
                nblk = 4 * m + 4
                for j in range(nblk):
                    jsl = slice(j * 128, (j + 1) * 128)
                    r = j - 4 * m
                    # band blocks only contribute to q columns >= 128*r
                    c0 = 128 * r if r > 0 else 0
                    cw = NQ - c0
                    for hh in range(2):
                        lo, hi = hh * 64, hh * 64 + 64
                        pss = ps_s.tile([128, NQ], FP32, name="pss", tag="ss")
                        nc.tensor.matmul(
                            pss[:, c0:NQ], lhsT=kt_sb[lo:hi, p, jsl],
                            rhs=qt_sb[lo:hi, p,
                                      m * NQ + c0:m * NQ + NQ],
                            start=True, stop=True,
                            tile_position=(hh * 64, 0))
                        pe = pexp_pool.tile([128, NQ], BF16)
                        nc.scalar.activation(pe[:, c0:NQ], pss[:, c0:NQ],
                                             AF.Exp)
                        if r >= 0:
                            # mask the 128-wide triangular strip in place
                            nc.gpsimd.tensor_mul(
                                pe[:, c0:c0 + 128], pe[:, c0:c0 + 128],
                                tri[:])
                        nc.tensor.matmul(
                            psa[hh][:, c0:NQ],
                            lhsT=v_sb[:, j, 2 * p + hh, :],
                            rhs=pe[:, c0:NQ],
                            start=(j == 0), stop=(j == nblk - 1))
                # early-evict raw A and denominators; psa banks free up fast
                for hh in range(2):
                    h = 2 * p + hh
                    nc.scalar.copy(coll[32 * h:32 * h + 1, :],
                                   psa[hh][64:65, :])
                    nc.scalar.copy(araw[h][:], psa[hh][0:64, :])
                # per-pair reciprocal on DVE: pair 0's overlaps pair 1's
                # attention; bcast+mult are deferred past the QKV hoist so
                # the PE queue never parks behind the (mis-modeled) recip
                pr = slice(64 * p, 64 * p + 33)
                nc.vector.reciprocal(recf[pr, :], coll[pr, :])
                nc.scalar.copy(recb[pr, :], recf[pr, :])
            # hoist next macro's projections: PE-dense filler while DVE
            # finishes the reciprocals
            if m + 1 < QM:
                qkv_stage(m + 1)
            for h in range(HPC):
                p_, hh = h // 2, h % 2
                lo, hi = hh * 64, hh * 64 + 64
                psb = ps_qkv.tile([128, NQ], FP32, name="psb", tag="qkv")
                nc.tensor.matmul(
                    psb[0:64, :],
                    lhsT=ones97[32 * h:32 * h + 1, :],
                    rhs=recb[32 * h:32 * h + 1, :],
                    start=True, stop=True,
                    tile_position=(32 * h, 0))
                nc.vector.tensor_mul(
                    as_sb[lo:hi, p_, msl], psb[0:64, :], araw[h][:])
            # projection for this q-macro (needs both pairs)
            for t in range(4):
                tsl = slice(m * NQ + t * 128, m * NQ + (t + 1) * 128)
                for e in range(2):
                    esl = slice(e * 512, (e + 1) * 512)
                    pso = ps_qkv.tile([128, 512], FP32, name="pso", tag="qkv")
                    nc.tensor.matmul(
                        pso[:], lhsT=as_sb[:, 0, tsl], rhs=wp_sb[:, 0, esl],
                        start=True, stop=False)
                    nc.tensor.matmul(
                        pso[:], lhsT=as_sb[:, 1, tsl], rhs=wp_sb[:, 1, esl],
                        start=False, stop=True)
                    osb = stage.tile([128, 512], FP32)
                    nc.vector.tensor_copy(osb[:], pso[:])
                    nc.sync.dma_start(out[tsl, esl], osb[:])

    nc.compile()
    return nc


_CACHED = {}


def _get_graph():
    if "nc" not in _CACHED:
        _CACHED["nc"] = build_graph()
    return _CACHED["nc"]


def make_in_maps(hidden_states, W_qkv, b_qkv, W_proj):
    bf16 = ml_dtypes.bfloat16
    in_maps = []
    xTb = [np.ascontiguousarray(hidden_states[b].T).astype(bf16)
           for b in range(B)]
    for c in range(NCORES):
        b = c // 4
        h0 = HPC * (c % 4)
        csl = slice(h0 * D, (h0 + HPC) * D)
        wq_s = np.ascontiguousarray(W_qkv[:, csl]).astype(bf16)
        wk_s = np.ascontiguousarray(W_qkv[:, E:][:, csl]).astype(bf16)
        wv_s = np.ascontiguousarray(W_qkv[:, 2 * E:][:, csl]).astype(bf16)
        bq = b_qkv[csl].reshape(HPC, D).T.astype(np.float32) / 8.0
        bk = b_qkv[E:][csl].reshape(HPC, D).T.astype(np.float32)
        qkbias = np.ascontiguousarray(
            np.concatenate([bq, bk], axis=0))          # (128, 4)
        bv = np.ascontiguousarray(
            b_qkv[2 * E:][csl].reshape(1, HPC * D)).astype(bf16)
        wp_s = np.ascontiguousarray(W_proj[csl, :]).astype(bf16)
        in_maps.append({
            "xT": xTb[b], "wq": wq_s, "wk": wk_s, "wv": wv_s,
            "qkbias": qkbias, "bv": bv, "wp": wp_s,
        })
    return in_maps


def kernel(hidden_states, W_qkv, b_qkv, W_proj, b_proj):
    from concourse.bass_utils import run_bass_kernel_spmd

    hidden_states = np.asarray(hidden_states, dtype=np.float32)
    W_qkv = np.asarray(W_qkv, dtype=np.float32)
    b_qkv = np.asarray(b_qkv, dtype=np.float32)
    W_proj = np.asarray(W_proj, dtype=np.float32)
    b_proj = np.asarray(b_proj, dtype=np.float32)

    nc = _get_graph()
    in_maps = make_in_maps(hidden_states, W_qkv, b_qkv, W_proj)
    res = run_bass_kernel_spmd(nc, in_maps, list(range(NCORES)))
    partials = [res.results[c]["out"] for c in range(NCORES)]
    outp = np.empty((B, S, E), dtype=np.float32)
    for b in range(B):
        acc = np.zeros((S, E), dtype=np.float64)
        for c in range(4 * b, 4 * b + 4):
            acc += partials[c].astype(np.float64)
        outp[b] = (acc + b_proj).astype(np.float32)
    return outp


# revision 25
# speedup vs baseline: 1.1574x; 1.1574x over previous
"""Causal multi-head attention on 8 Trainium2 NeuronCores.

Problem: B=2, S=2048, E=1024, H=16 heads, D=64.
Sharding: core c handles batch b = c // 4 and heads [4*(c%4) .. 4*(c%4)+3]
(data parallel on B x tensor parallel on heads). Each core computes its
partial output projection; the host sums the 4 partials per batch and adds
b_proj (the standard row-parallel TP reduction, done on host).

Per-core kernel (all matmuls bf16 inputs, fp32 PSUM accumulation), computed
in "transposed" space to avoid transposing softmax probabilities:
  xT = X[b]^T in SBUF.
  Q^T/K^T [D, S] per head via col-packed matmuls (Q in psum rows 0:64, K in
  rows 64:128), evicted with ACT (bias + 1/8 scaling folded into Q).
  V [S, D] per head with a ones column appended -> AV matmul also
  accumulates softmax denominators.
  Scores S^T [k, q] = K^T.T @ Q^T, row-packed across head pairs.
  P^T = exp(S^T) on ACT; diagonal blocks masked with precomputed causal
  masks; strictly-upper blocks skipped.
  A^T [65, q] = V_ext.T @ P^T accumulated over k tiles (row 64 = denom).
  1/denom broadcast across partitions via a K=1 outer-product matmul,
  divide on DVE into head-pair-stacked A_scaled^T.
  partial[q, e] = A_scaled^T.T @ W_proj_rows accumulated over both pairs.
"""

import os
import sys
from contextlib import ExitStack

for _p in ("/opt/trn_rl_repo", "/root/.axon_site/_ro/trn_rl_repo"):
    if os.path.isdir(_p) and _p not in sys.path:
        sys.path.append(_p)

import numpy as np
import ml_dtypes

import concourse.bass as bass
import concourse.tile as tile
from concourse import bacc
from concourse import mybir
from concourse.masks import make_identity  # noqa: F401  (import check)

FP32 = mybir.dt.float32
BF16 = mybir.dt.bfloat16
AF = mybir.ActivationFunctionType

B, S, E, H = 2, 2048, 1024, 16
D = E // H          # 64
NCORES = 8
HPC = 4             # heads per core
NPAIR = 2           # head pairs per core
KT = E // 128       # 8 contraction tiles over E
ST = S // 128       # 16 tiles over S (k dimension)
QM = S // 512       # 4 q-macro tiles of 512
NQ = 512


def build_graph():
    nc = bacc.Bacc()

    xT = nc.declare_dram_parameter("xT", [E, S], BF16, isOutput=False)
    wq = nc.declare_dram_parameter("wq", [E, HPC * D], BF16, isOutput=False)
    wk = nc.declare_dram_parameter("wk", [E, HPC * D], BF16, isOutput=False)
    wv = nc.declare_dram_parameter("wv", [E, HPC * D], BF16, isOutput=False)
    qkbias = nc.declare_dram_parameter("qkbias", [128, HPC], FP32, isOutput=False)
    bv = nc.declare_dram_parameter("bv", [1, HPC * D], BF16, isOutput=False)
    wp = nc.declare_dram_parameter("wp", [HPC * D, E], BF16, isOutput=False)
    out = nc.declare_dram_parameter("out", [S, E], FP32, isOutput=True)

    with tile.TileContext(nc) as tc, ExitStack() as ctx:
        const = ctx.enter_context(tc.tile_pool(name="const", bufs=1))
        sb = ctx.enter_context(tc.tile_pool(name="sb", bufs=1))
        pexp_pool = ctx.enter_context(tc.tile_pool(name="pexp", bufs=8))
        stage = ctx.enter_context(tc.tile_pool(name="stage", bufs=3))
        rec_pool = ctx.enter_context(tc.tile_pool(name="rec", bufs=2))
        araw_pool = ctx.enter_context(tc.tile_pool(name="araw", bufs=2))

        # PSUM budget is 8 banks: scores 2x2-bank + qkv/proj/bcast 2 + psa 2
        ps_s = ctx.enter_context(tc.tile_pool(name="ps_s", bufs=2, space="PSUM"))
        ps_qkv = ctx.enter_context(tc.tile_pool(name="ps_qkv", bufs=2, space="PSUM"))
        ps_a = ctx.enter_context(tc.tile_pool(name="ps_a", bufs=1, space="PSUM"))

        # ---- persistent SBUF tensors ----
        xt_sb = sb.tile([128, KT, S], BF16)          # X^T tiles, kt-major
        qt_sb = sb.tile([128, NPAIR, S], BF16)       # Q^T, pair-stacked
        kt_sb = sb.tile([128, NPAIR, S], BF16)       # K^T, pair-stacked
        v_sb = sb.tile([128, ST, HPC, D + 1], BF16)  # [V | ones] per ktile/head
        as_sb = sb.tile([128, NPAIR, S], BF16)       # A_scaled^T, pair-stacked
        wq_sb = sb.tile([128, KT, HPC * D], BF16)
        wk_sb = sb.tile([128, KT, HPC * D], BF16)
        wv_sb = sb.tile([128, KT, HPC * D], BF16)
        wp_sb = sb.tile([128, NPAIR, E], BF16)
        qkb_sb = const.tile([128, HPC], FP32)
        bv_sb = const.tile([1, HPC * D], BF16)
        ones_row = const.tile([1, 128], BF16)        # K=1 lhsT for V bias
        ones97 = const.tile([97, D], BF16)           # rows 0/32/64/96: K=1 lhsT
        tri = const.tile([128, 128], BF16)           # upper-tri (incl diag) strip mask

        # ---- constants ----
        nc.any.memset(ones_row[:], 1.0)
        for _h in range(4):
            nc.any.memset(ones97[32 * _h:32 * _h + 1, :], 1.0)
        nc.any.memset(v_sb[:, :, :, D:D + 1], 1.0)
        # tri[kk, qq] = 1 where kk <= qq else 0
        nc.any.memset(tri[:], 1.0)
        nc.gpsimd.affine_select(
            out=tri[:], in_=tri[:],
            compare_op=mybir.AluOpType.is_ge, fill=0.0,
            base=0, pattern=[[1, 128]], channel_multiplier=-1)

        # ---- input DMAs: critical-first, alternating the two HWDGE rings ----
        _dq = [nc.sync, nc.scalar]
        _di = [0]

        def dma_in(dst, src):
            _dq[_di[0] % 2].dma_start(dst, src)
            _di[0] += 1

        dma_in(qkb_sb[:], qkbias[:])
        dma_in(bv_sb[:], bv[:])
        for kt in range(KT):
            dma_in(wv_sb[:, kt, :], wv[kt * 128:(kt + 1) * 128, :])
            dma_in(xt_sb[:, kt, 0:NQ], xT[kt * 128:(kt + 1) * 128, 0:NQ])
        for kt in range(KT):
            dma_in(wq_sb[:, kt, :], wq[kt * 128:(kt + 1) * 128, :])
            dma_in(wk_sb[:, kt, :], wk[kt * 128:(kt + 1) * 128, :])
        for _c in range(1, QM):
            for kt in range(KT):
                dma_in(xt_sb[:, kt, _c * NQ:(_c + 1) * NQ],
                       xT[kt * 128:(kt + 1) * 128, _c * NQ:(_c + 1) * NQ])
            if _c == 1:
                dma_in(wp_sb[:, 0, :], wp[0:128, :])
                dma_in(wp_sb[:, 1, :], wp[128:256, :])

        def qkv_stage(m):
            msl = slice(m * NQ, (m + 1) * NQ)
            # V projection for this macro's 4 s-tiles
            for st in range(4 * m, 4 * m + 4):
                psv = ps_qkv.tile([128, NQ], FP32, name="psv", tag="qkv")
                ssl = slice(st * 128, (st + 1) * 128)
                for kt in range(KT):
                    nc.tensor.matmul(
                        psv[:, 0:HPC * D], lhsT=xt_sb[:, kt, ssl],
                        rhs=wv_sb[:, kt, :], start=(kt == 0), stop=False)
                nc.tensor.matmul(
                    psv[:, 0:HPC * D], lhsT=ones_row[:], rhs=bv_sb[:],
                    start=False, stop=True)
                nc.vector.tensor_copy(
                    v_sb[:, st, :, 0:D],
                    psv[:, 0:HPC * D].rearrange("p (h d) -> p h d", h=HPC))
            # Q^T / K^T for this macro, col-packed (Q rows 0:64, K rows 64:128)
            for p in range(NPAIR):
                for hh in range(2):
                    h = 2 * p + hh
                    lo, hi = hh * 64, hh * 64 + 64
                    psqk = ps_qkv.tile([128, NQ], FP32, name="psqk", tag="qkv")
                    for kt in range(KT):
                        nc.tensor.matmul(
                            psqk[0:64, :],
                            lhsT=wq_sb[:, kt, h * D:(h + 1) * D],
                            rhs=xt_sb[:, kt, msl],
                            start=(kt == 0), stop=(kt == KT - 1),
                            tile_position=(0, 0), skip_group_check=True)
                        nc.tensor.matmul(
                            psqk[64:128, :],
                            lhsT=wk_sb[:, kt, h * D:(h + 1) * D],
                            rhs=xt_sb[:, kt, msl],
                            start=(kt == 0), stop=(kt == KT - 1),
                            tile_position=(0, 64), skip_group_check=True)
                    nc.vector.tensor_scalar(
                        qt_sb[lo:hi, p, msl], psqk[0:64, :],
                        0.125, qkb_sb[0:64, h:h + 1],
                        op0=mybir.AluOpType.mult, op1=mybir.AluOpType.add)
                    nc.vector.tensor_scalar_add(
                        kt_sb[lo:hi, p, msl], psqk[64:128, :],
                        qkb_sb[64:128, h:h + 1])
        qkv_stage(0)
        for m in range(QM):
            msl = slice(m * NQ, (m + 1) * NQ)
            coll = rec_pool.tile([97, NQ], FP32, name="coll", tag="coll")
            nc.any.memset(coll[:], 1.0)
            recf = rec_pool.tile([97, NQ], FP32, name="recf", tag="recf")
            recb = rec_pool.tile([97, NQ], BF16, name="recb", tag="recb")
            araw = [araw_pool.tile([64, NQ], BF16, name=f"araw{_h}",
                                   tag=f"araw{_h}") for _h in range(HPC)]
            for p in range(NPAIR):
                psa = [ps_a.tile([65, NQ], FP32, name=f"psa{_hh}",
                                 tag=f"psa{_hh}")
                       for _hh in range(2)]
                nblk = 4 * m + 4
                for j in range(nblk):
                    jsl = slice(j * 128, (j + 1) * 128)
                    r = j - 4 * m
                    # band blocks only contribute to q columns >= 128*r
                    c0 = 128 * r if r > 0 else 0
                    # both heads' scores into one 2-bank psum tile
                    pss = ps_s.tile([128, 2 * NQ], FP32, name="pss", tag="ss")
                    pe = pexp_pool.tile([128, 2 * NQ], BF16)
                    for hh in range(2):
                        lo, hi = hh * 64, hh * 64 + 64
                        nc.tensor.matmul(
                            pss[:, hh * NQ + c0:hh * NQ + NQ],
                            lhsT=kt_sb[lo:hi, p, jsl],
                            rhs=qt_sb[lo:hi, p,
                                      m * NQ + c0:m * NQ + NQ],
                            start=True, stop=True,
                            tile_position=(hh * 64, 0))
                    # one exp for both heads (halves ACT per-op overhead)
                    nc.scalar.activation(
                        pe[:].rearrange("p (g q) -> p g q", g=2)[:, :, c0:NQ],
                        pss[:].rearrange("p (g q) -> p g q", g=2)[:, :, c0:NQ],
                        AF.Exp)
                    for hh in range(2):
                        if r >= 0:
                            # mask the 128-wide triangular strip in place
                            nc.gpsimd.tensor_mul(
                                pe[:, hh * NQ + c0:hh * NQ + c0 + 128],
                                pe[:, hh * NQ + c0:hh * NQ + c0 + 128],
                                tri[:])
                        nc.tensor.matmul(
                            psa[hh][:, c0:NQ],
                            lhsT=v_sb[:, j, 2 * p + hh, :],
                            rhs=pe[:, hh * NQ + c0:hh * NQ + NQ],
                            start=(j == 0), stop=(j == nblk - 1))
                # early-evict raw A and denominators; psa banks free up fast
                for hh in range(2):
                    h = 2 * p + hh
                    nc.scalar.copy(coll[32 * h:32 * h + 1, :],
                                   psa[hh][64:65, :])
                    nc.scalar.copy(araw[h][:], psa[hh][0:64, :])
                # per-pair reciprocal on DVE: pair 0's overlaps pair 1's
                # attention; bcast+mult are deferred past the QKV hoist so
                # the PE queue never parks behind the (mis-modeled) recip
                pr = slice(64 * p, 64 * p + 33)
                nc.vector.reciprocal(recf[pr, :], coll[pr, :])
                nc.scalar.copy(recb[pr, :], recf[pr, :])
            # hoist next macro's projections: PE-dense filler while DVE
            # finishes the reciprocals
            if m + 1 < QM:
                qkv_stage(m + 1)
            for h in range(HPC):
                p_, hh = h // 2, h % 2
                lo, hi = hh * 64, hh * 64 + 64
                psb = ps_qkv.tile([128, NQ], FP32, name="psb", tag="qkv")
                nc.tensor.matmul(
                    psb[0:64, :],
                    lhsT=ones97[32 * h:32 * h + 1, :],
                    rhs=recb[32 * h:32 * h + 1, :],
                    start=True, stop=True,
                    tile_position=(32 * h, 0))
                nc.vector.tensor_mul(
                    as_sb[lo:hi, p_, msl], psb[0:64, :], araw[h][:])
            # projection for this q-macro (needs both pairs)
            for t in range(4):
                tsl = slice(m * NQ + t * 128, m * NQ + (t + 1) * 128)
                for e in range(2):
                    esl = slice(e * 512, (e + 1) * 512)
                    pso = ps_qkv.tile([128, 512], FP32, name="pso", tag="qkv")
                    nc.tensor.matmul(
                        pso[:], lhsT=as_sb[:, 0, tsl], rhs=wp_sb[:, 0, esl],
                        start=True, stop=False)
                    nc.tensor.matmul(
                        pso[:], lhsT=as_sb[:, 1, tsl], rhs=wp_sb[:, 1, esl],
                        start=False, stop=True)
                    osb = stage.tile([128, 512], FP32)
                    nc.vector.tensor_copy(osb[:], pso[:])
                    nc.sync.dma_start(out[tsl, esl], osb[:])

    nc.compile()
    return nc


_CACHED = {}


def _get_graph():
    if "nc" not in _CACHED:
        _CACHED["nc"] = build_graph()
    return _CACHED["nc"]


def make_in_maps(hidden_states, W_qkv, b_qkv, W_proj):
    bf16 = ml_dtypes.bfloat16
    in_maps = []
    xTb = [np.ascontiguousarray(hidden_states[b].T).astype(bf16)
           for b in range(B)]
    for c in range(NCORES):
        b = c // 4
        h0 = HPC * (c % 4)
        csl = slice(h0 * D, (h0 + HPC) * D)
        wq_s = np.ascontiguousarray(W_qkv[:, csl]).astype(bf16)
        wk_s = np.ascontiguousarray(W_qkv[:, E:][:, csl]).astype(bf16)
        wv_s = np.ascontiguousarray(W_qkv[:, 2 * E:][:, csl]).astype(bf16)
        bq = b_qkv[csl].reshape(HPC, D).T.astype(np.float32) / 8.0
        bk = b_qkv[E:][csl].reshape(HPC, D).T.astype(np.float32)
        qkbias = np.ascontiguousarray(
            np.concatenate([bq, bk], axis=0))          # (128, 4)
        bv = np.ascontiguousarray(
            b_qkv[2 * E:][csl].reshape(1, HPC * D)).astype(bf16)
        wp_s = np.ascontiguousarray(W_proj[csl, :]).astype(bf16)
        in_maps.append({
            "xT": xTb[b], "wq": wq_s, "wk": wk_s, "wv": wv_s,
            "qkbias": qkbias, "bv": bv, "wp": wp_s,
        })
    return in_maps


def kernel(hidden_states, W_qkv, b_qkv, W_proj, b_proj):
    from concourse.bass_utils import run_bass_kernel_spmd

    hidden_states = np.asarray(hidden_states, dtype=np.float32)
    W_qkv = np.asarray(W_qkv, dtype=np.float32)
    b_qkv = np.asarray(b_qkv, dtype=np.float32)
    W_proj = np.asarray(W_proj, dtype=np.float32)
    b_proj = np.asarray(b_proj, dtype=np.float32)

    nc = _get_graph()
    in_maps = make_in_maps(hidden_states, W_qkv, b_qkv, W_proj)
    res = run_bass_kernel_spmd(nc, in_maps, list(range(NCORES)))
    partials = [res.results[c]["out"] for c in range(NCORES)]
    outp = np.empty((B, S, E), dtype=np.float32)
    for b in range(B):
        acc = np.zeros((S, E), dtype=np.float64)
        for c in range(4 * b, 4 * b + 4):
            acc += partials[c].astype(np.float64)
        outp[b] = (acc + b_proj).astype(np.float32)
    return outp
